# revision 1
# baseline (speedup 1.0000x reference)
"""Trainium2 kernel for nn_BFM_torch_56384330662315 (gnn_message_passing).

Reference semantics (B=4, C=128, N=2048, K=16):
  feats = transpose(seg_features, (0,2,1))                 # [B,N,C]
  per sample: adj = boundary-cut symmetric kNN graph; two GCN layers
  out = refined + feats

Each GCN layer computes ``out = (adj/deg) @ f + feat`` but returns plain
``feat`` whenever any node has zero degree (``has_zero`` in the reference).
Any node classified as a boundary node (argmax(edge_preds)==1) gets its row
AND column zeroed in the symmetric adjacency, so it has zero degree.  Hence
for every sample that has at least one edge node and at least one non-edge
node, both GCN layers are exact identities and the whole module reduces,
bit-for-bit in fp32, to:

  out = 2 * transpose(seg_features, (0,2,1))

The device kernel evaluates that scaled transpose, data-parallel over
8 NeuronCores (each core transposes a [128, 1024] half-sample via the PE
transpose path).  The per-sample condition is checked on host from
edge_preds (tiny); samples that don't satisfy it (probability ~2^-2047
for the randn inputs this problem is generated with) fall back to an
exact numpy port of the reference.
"""

import numpy as np

B, C, N, K = 4, 128, 2048, 16
GEO_FILL = 1000.0
NCORES = 8
NSH = N * B // NCORES  # 1024 columns of seg_features per core
NBLK = NSH // 128      # 8 transpose blocks per core

# test.py can flip this on to collect an NTFF hardware profile.
TRACE = False
LAST_RESULTS = None

_COMPILED = None


def _build_nc():
    """Per-core program: y[NSH, C] = 2 * x[C, NSH]^T via PE transpose."""
    import concourse.bass as bass
    import concourse.tile as tile
    from concourse import bacc, mybir

    nc = bacc.Bacc(
        "TRN2",
        target_bir_lowering=False,
        debug=False,
        num_devices=NCORES,
    )
    x = nc.dram_tensor("x", [C, NSH], mybir.dt.float32, kind="ExternalInput").ap()
    ident = nc.dram_tensor(
        "ident", [128, 128], mybir.dt.float32, kind="ExternalInput"
    ).ap()
    y = nc.dram_tensor("y", [NSH, C], mybir.dt.float32, kind="ExternalOutput").ap()

    with tile.TileContext(nc) as tc:
        with (
            tc.tile_pool(name="const", bufs=1) as cpool,
            tc.tile_pool(name="xin", bufs=4) as ipool,
            tc.tile_pool(name="ps", bufs=4, space="PSUM") as ppool,
            tc.tile_pool(name="yout", bufs=4) as opool,
        ):
            idt = cpool.tile([128, 128], mybir.dt.float32)
            nc.sync.dma_start(idt[:], ident[:])
            for i in range(NBLK):
                xt = ipool.tile([128, 128], mybir.dt.float32, tag="xt")
                nc.sync.dma_start(xt[:], x[:, bass.ts(i, 128)])
                ps = ppool.tile([128, 128], mybir.dt.float32, tag="ps")
                nc.tensor.transpose(ps[:], xt[:], idt[:])
                ot = opool.tile([128, 128], mybir.dt.float32, tag="ot")
                nc.scalar.mul(ot[:], ps[:], 2.0)
                nc.sync.dma_start(y[bass.ts(i, 128), :], ot[:])
    nc.compile()
    return nc


def _run_device(seg: np.ndarray) -> np.ndarray:
    """seg [B,C,N] f32 -> 2*transpose [B,N,C], sharded half-sample per core."""
    global _COMPILED, LAST_RESULTS
    from concourse.bass_utils import run_bass_kernel_spmd

    if _COMPILED is None:
        _COMPILED = _build_nc()
    nc = _COMPILED

    ident = np.eye(128, dtype=np.float32)
    in_maps = []
    for k in range(NCORES):
        b, h = k // 2, k % 2
        shard = np.ascontiguousarray(seg[b, :, h * NSH : (h + 1) * NSH])
        in_maps.append({"x": shard, "ident": ident})

    res = run_bass_kernel_spmd(nc, in_maps, list(range(NCORES)), trace=TRACE)
    LAST_RESULTS = res

    out = np.empty((B, N, C), dtype=np.float32)
    for k in range(NCORES):
        b, h = k // 2, k % 2
        out[b, h * NSH : (h + 1) * NSH, :] = np.asarray(
            res.results[k]["y"]
        ).reshape(NSH, C)
    return out


# ---------------------------------------------------------------------------
# Exact numpy port of the reference — fallback for samples where the GCN does
# not collapse to identity (never hit for this problem's input distribution).
# ---------------------------------------------------------------------------


def _np_build_adj(g, edge_cls, k):
    n = g.shape[0]
    nbrs = np.argsort(g, axis=-1, kind="stable")[:, :k]
    rows = np.arange(n)[:, None]
    adj = np.zeros((n, n), g.dtype)
    adj[rows, nbrs] = 1.0
    adj[nbrs, rows] = 1.0
    is_edge = edge_cls == 1
    adj = np.where(is_edge[:, None], 0.0, adj)
    edge_col = is_edge[None, :]
    cond = (adj == 1) & edge_col
    maxgeo = np.min(np.where(cond, g, GEO_FILL), axis=-1)
    adjr = np.where(g > maxgeo[:, None], 0.0, adj)
    adjr = np.where(edge_col, 0.0, adjr)
    adj2 = np.where(is_edge[:, None], 0.0, adjr)
    adj_sym = ((adj2 > 0) | (adj2.T > 0)).astype(g.dtype)
    if np.all(is_edge):
        return np.eye(n, dtype=g.dtype)
    return adj_sym


def _np_gcn(feat, adj, W, b):
    identity = feat
    f = np.maximum(feat @ W.T + b, 0.0).astype(np.float32)
    row_deg = np.sum(adj, axis=-1, keepdims=True)
    col_deg = np.sum(adj, axis=-2, keepdims=True)
    degree = np.sqrt(row_deg) @ np.sqrt(col_deg)
    if np.any(degree == 0):
        return identity
    out = (adj / degree) @ f + identity
    return out.astype(np.float32)


def _np_sample(feat, ep, g, W1, b1, W2, b2):
    edge_cls = np.argmax(ep, axis=0)
    adj = _np_build_adj(g, edge_cls, K)
    r = _np_gcn(feat, adj, W1, b1)
    r = _np_gcn(r, adj, W2, b2)
    return r


def kernel(**inputs) -> np.ndarray:
    seg = np.ascontiguousarray(np.asarray(inputs["seg_features"], dtype=np.float32))
    ep = np.asarray(inputs["edge_preds"], dtype=np.float32)

    # argmax over the 2 class logits: class 1 iff ep[1] > ep[0] (ties -> 0)
    edge = ep[:, 1, :] > ep[:, 0, :]
    any_e = edge.any(axis=1)
    all_e = edge.all(axis=1)
    fast = any_e & ~all_e  # GCN layers are exact identities

    out = _run_device(seg)  # 2 * transpose, correct wherever fast[b]

    if not fast.all():
        g_all = np.asarray(inputs["gmatrix"], dtype=np.float32)
        W1 = np.asarray(inputs["W1"], dtype=np.float32)
        b1 = np.asarray(inputs["b1"], dtype=np.float32)
        W2 = np.asarray(inputs["W2"], dtype=np.float32)
        b2 = np.asarray(inputs["b2"], dtype=np.float32)
        for b in range(B):
            if not fast[b]:
                feat = np.ascontiguousarray(seg[b].T)
                r = _np_sample(feat, ep[b], g_all[b], W1, b1, W2, b2)
                out[b] = r + feat
    return out
